# revision 7
# baseline (speedup 1.0000x reference)
"""Energy-based debias loss kernel for Trainium2 (8 NeuronCores, Bass/Tile).

Math (per row i of logits [N, C], with uniform noise U, class bias cb):
    S_i    = sum_j exp(L_ij)                      (logsumexp, no max-sub needed:
                                                   L ~ N(0,1), sums are safe fp32)
    lse_i  = ln(S_i)
    S'_i   = S_i - exp(L_it)                      (exclude-target sum)
    blse_i = ln(S'_i)
    beta_i = relu(blse_i) / lse_i                 (== where(-blse>0,0,-blse)/(-lse))
    g_ij   = -ln(-ln(U_ij + 1e-10) + 1e-10)       (gumbel from uniform)
    z_ij   = L_ij + beta_i * g_ij + ln(cb_j + 1e-12)
    nll_i  = ln(sum_j exp(z_ij)) - z_it
    loss   = mean_i nll_i

Engine mapping per 128-row block (rows on partitions, C streamed in chunks):
  pass 1 (per chunk):  ACT exp(L) with fused accum_out -> S partial sums
                       PE  ones[1,128] x lb[1,ck] -> PSUM broadcast of ln(cb)
                       DVE L += lb  (PSUM operand)
  pass 1.5 (tiny):     beta chain on [128,1] tiles; target gathers done upfront
                       via indirect DMA
  pass 2 (per chunk):  ACT ln(U+eps); ACT ln(-a+eps); DVE scalar_tensor_tensor
                       w = b*(-beta) + (L+lb); ACT exp(w) with fused accum -> S2
No standalone reduce passes; row sums ride the ACT accumulator.
"""

import numpy as np

import concourse.bass as bass
import concourse.bacc as bacc
import concourse.tile as tile
from concourse import mybir
from concourse.bass_utils import run_bass_kernel_spmd

P = 128
N_CORES = 8

# tunables
CK = 3200          # chunk size along C
L_BUFS_EXTRA = 0   # extra L slots beyond nch for cross-block overlap
U_BUFS = 3
MM_N = 512         # psum bank free size for the broadcast matmul

F32 = mybir.dt.float32
BF16 = mybir.dt.bfloat16
I32 = mybir.dt.int32
AF = mybir.ActivationFunctionType
ALU = mybir.AluOpType


def build_nc(R, C, ck=CK):
    """Build the SPMD per-core program. R rows per core, C classes."""
    assert R % P == 0 and C % ck == 0 and C % P == 0
    nblk = R // P
    nch = C // ck
    ckb = C // P  # free size of the [128, C/128] class-bias view

    nc = bacc.Bacc(None, target_bir_lowering=False, debug=False)

    logits_in = nc.dram_tensor("logits", [R, C], F32, kind="ExternalInput")
    u_in = nc.dram_tensor("u", [R, C], F32, kind="ExternalInput")
    tidx_in = nc.dram_tensor("tidx", [R], I32, kind="ExternalInput")  # i*C+t_i
    tgt_in = nc.dram_tensor("tgt", [R], I32, kind="ExternalInput")    # t_i
    cb_in = nc.dram_tensor("cb", [C], F32, kind="ExternalInput")
    nll_out = nc.dram_tensor("nll", [P, nblk], F32, kind="ExternalOutput")
    lb_dram = nc.dram_tensor("lb_bf16", [C], BF16)  # internal scratch

    logits_flat = logits_in[:].rearrange("r c -> (r c)").unsqueeze(1)
    u_flat = u_in[:].rearrange("r c -> (r c)").unsqueeze(1)
    cb_flat = cb_in[:].unsqueeze(1)

    with tile.TileContext(nc) as tc:
        with (
            tc.tile_pool(name="consts", bufs=1) as consts,
            tc.tile_pool(name="Lpool", bufs=nch + L_BUFS_EXTRA) as lpool,
            tc.tile_pool(name="Upool", bufs=U_BUFS) as upool,
            tc.tile_pool(name="scr", bufs=1) as scrpool,
            tc.tile_pool(name="lbk", bufs=2) as lbkpool,
            tc.tile_pool(name="stats", bufs=4) as stats,
            tc.tile_pool(name="smalls", bufs=24) as smalls,
            tc.tile_pool(name="psum", bufs=1, space="PSUM") as psum,
        ):
            # ---- phase 0: constants, ln(class_bias), upfront gathers ----
            eps10 = consts.tile([P, 1], F32)  # 1e-10 bias for Ln
            nc.vector.memset(eps10[:], 1e-10)
            eps12 = consts.tile([P, 1], F32)  # 1e-12 bias for Ln(class_bias)
            nc.vector.memset(eps12[:], 1e-12)

            cb_t = consts.tile([P, ckb], F32)
            nc.sync.dma_start(out=cb_t[:], in_=cb_in[:].rearrange("(p k) -> p k", p=P))
            lb128 = consts.tile([P, ckb], BF16)
            nc.scalar.activation(out=lb128[:], in_=cb_t[:], func=AF.Ln, bias=eps12[:])
            nc.sync.dma_start(
                out=lb_dram[:].rearrange("(p k) -> p k", p=P), in_=lb128[:]
            )

            ones_bf = consts.tile([1, P], BF16)
            nc.vector.memset(ones_bf[:], 1.0)

            tidx_sb = consts.tile([P, nblk], I32)
            nc.sync.dma_start(
                out=tidx_sb[:], in_=tidx_in[:].rearrange("(b p) -> p b", p=P)
            )
            tgt_sb = consts.tile([P, nblk], I32)
            nc.sync.dma_start(
                out=tgt_sb[:], in_=tgt_in[:].rearrange("(b p) -> p b", p=P)
            )

            xt_all = consts.tile([P, nblk], F32)   # logits[i, t_i]
            ut_all = consts.tile([P, nblk], F32)   # U[i, t_i]
            cbt_all = consts.tile([P, nblk], F32)  # cb[t_i]
            for b in range(nblk):
                nc.gpsimd.indirect_dma_start(
                    out=xt_all[:, b : b + 1],
                    out_offset=None,
                    in_=logits_flat,
                    in_offset=bass.IndirectOffsetOnAxis(
                        ap=tidx_sb[:, b : b + 1], axis=0
                    ),
                )
                nc.gpsimd.indirect_dma_start(
                    out=ut_all[:, b : b + 1],
                    out_offset=None,
                    in_=u_flat,
                    in_offset=bass.IndirectOffsetOnAxis(
                        ap=tidx_sb[:, b : b + 1], axis=0
                    ),
                )
                nc.gpsimd.indirect_dma_start(
                    out=cbt_all[:, b : b + 1],
                    out_offset=None,
                    in_=cb_flat,
                    in_offset=bass.IndirectOffsetOnAxis(
                        ap=tgt_sb[:, b : b + 1], axis=0
                    ),
                )

            # target-side tiny precomputes (block independent)
            eT_all = consts.tile([P, nblk], F32)
            nc.scalar.activation(out=eT_all[:], in_=xt_all[:], func=AF.Exp)
            at_all = consts.tile([P, nblk], F32)
            nc.scalar.activation(out=at_all[:], in_=ut_all[:], func=AF.Ln, bias=eps10[:])
            bt_all = consts.tile([P, nblk], F32)
            nc.scalar.activation(
                out=bt_all[:], in_=at_all[:], func=AF.Ln, scale=-1.0, bias=eps10[:]
            )
            lbt_all = consts.tile([P, nblk], F32)
            nc.scalar.activation(out=lbt_all[:], in_=cbt_all[:], func=AF.Ln, bias=eps12[:])
            s1_all = consts.tile([P, nblk], F32)  # x_t + ln(cb_t)
            nc.vector.tensor_tensor(
                out=s1_all[:], in0=xt_all[:], in1=lbt_all[:], op=ALU.add
            )

            nll_sb = consts.tile([P, nblk], F32)

            for b in range(nblk):
                r0 = b * P
                s_part = stats.tile([P, nch], F32, tag="spart")
                s2_part = stats.tile([P, nch], F32, tag="s2part")
                l_tiles = []

                # ---- pass 1: S row-sums + fold ln(cb) into cached L ----
                for c in range(nch):
                    c0 = c * ck
                    lt = lpool.tile([P, ck], F32, tag="L")
                    l_tiles.append(lt)
                    nc.sync.dma_start(
                        out=lt[:], in_=logits_in[r0 : r0 + P, c0 : c0 + ck]
                    )
                    scr = scrpool.tile([P, ck], F32, tag="scr")
                    nc.scalar.activation(
                        out=scr[:],
                        in_=lt[:],
                        func=AF.Exp,
                        accum_out=s_part[:, c : c + 1],
                    )
                    lbk = lbkpool.tile([1, ck], BF16, tag="lbk")
                    nc.sync.dma_start(
                        out=lbk[:],
                        in_=lb_dram[c0 : c0 + ck].rearrange("(a c) -> a c", a=1),
                    )
                    lbb = psum.tile([P, ck], F32, tag="lbb")
                    for j in range(0, ck, MM_N):
                        n = min(MM_N, ck - j)
                        nc.tensor.matmul(
                            out=lbb[:, j : j + n],
                            lhsT=ones_bf[:],
                            rhs=lbk[:, j : j + n],
                            start=True,
                            stop=True,
                        )
                    nc.vector.tensor_tensor(
                        out=lt[:], in0=lt[:], in1=lbb[:], op=ALU.add
                    )

                # ---- pass 1.5: beta / z_t (tiny [128,1] chain) ----
                s_sum = smalls.tile([P, 1], F32, tag="sm")
                nc.vector.reduce_sum(
                    out=s_sum[:], in_=s_part[:], axis=mybir.AxisListType.X
                )
                lse = smalls.tile([P, 1], F32, tag="sm")
                nc.scalar.activation(out=lse[:], in_=s_sum[:], func=AF.Ln)
                sp = smalls.tile([P, 1], F32, tag="sm")
                nc.vector.tensor_tensor(
                    out=sp[:], in0=s_sum[:], in1=eT_all[:, b : b + 1], op=ALU.subtract
                )
                blse = smalls.tile([P, 1], F32, tag="sm")
                nc.scalar.activation(out=blse[:], in_=sp[:], func=AF.Ln)
                rb = smalls.tile([P, 1], F32, tag="sm")
                nc.scalar.activation(out=rb[:], in_=blse[:], func=AF.Relu)
                invl = smalls.tile([P, 1], F32, tag="sm")
                nc.vector.reciprocal(out=invl[:], in_=lse[:])
                beta = smalls.tile([P, 1], F32, tag="sm")
                nc.vector.tensor_tensor(out=beta[:], in0=rb[:], in1=invl[:], op=ALU.mult)
                negbeta = smalls.tile([P, 1], F32, tag="sm")
                nc.vector.tensor_scalar_mul(out=negbeta[:], in0=beta[:], scalar1=-1.0)
                znt = smalls.tile([P, 1], F32, tag="sm")  # beta * b_t
                nc.vector.tensor_tensor(
                    out=znt[:], in0=bt_all[:, b : b + 1], in1=beta[:], op=ALU.mult
                )
                zt = smalls.tile([P, 1], F32, tag="sm")  # z at target
                nc.vector.tensor_tensor(
                    out=zt[:], in0=s1_all[:, b : b + 1], in1=znt[:], op=ALU.subtract
                )

                # ---- pass 2: S2 = sum_j exp(L + lb - beta*b) ----
                for c in range(nch):
                    c0 = c * ck
                    ut = upool.tile([P, ck], F32, tag="U")
                    nc.sync.dma_start(out=ut[:], in_=u_in[r0 : r0 + P, c0 : c0 + ck])
                    nc.scalar.activation(
                        out=ut[:], in_=ut[:], func=AF.Ln, bias=eps10[:]
                    )
                    nc.scalar.activation(
                        out=ut[:], in_=ut[:], func=AF.Ln, scale=-1.0, bias=eps10[:]
                    )
                    nc.vector.scalar_tensor_tensor(
                        out=ut[:],
                        in0=ut[:],
                        scalar=negbeta[:],
                        op0=ALU.mult,
                        in1=l_tiles[c][:],
                        op1=ALU.add,
                    )
                    nc.scalar.activation(
                        out=ut[:],
                        in_=ut[:],
                        func=AF.Exp,
                        accum_out=s2_part[:, c : c + 1],
                    )

                s2_sum = smalls.tile([P, 1], F32, tag="sm")
                nc.vector.reduce_sum(
                    out=s2_sum[:], in_=s2_part[:], axis=mybir.AxisListType.X
                )
                l2 = smalls.tile([P, 1], F32, tag="sm")
                nc.scalar.activation(out=l2[:], in_=s2_sum[:], func=AF.Ln)
                nc.vector.tensor_tensor(
                    out=nll_sb[:, b : b + 1], in0=l2[:], in1=zt[:], op=ALU.subtract
                )

            nc.sync.dma_start(out=nll_out[:], in_=nll_sb[:])

    nc.finalize()  # runs Bacc.compile(): register alloc, act-table loads, ...
    return nc


_NC_CACHE = {}


def _get_nc(R, C, ck=CK):
    key = (R, C, ck)
    if key not in _NC_CACHE:
        _NC_CACHE[key] = build_nc(R, C, ck)
    return _NC_CACHE[key]


def make_in_maps(logits, targets, U, class_bias, n_cores=N_CORES):
    N, C = logits.shape
    R = N // n_cores
    in_maps = []
    for k in range(n_cores):
        sl = slice(k * R, (k + 1) * R)
        t = np.asarray(targets[sl], dtype=np.int64)
        in_maps.append(
            {
                "logits": np.ascontiguousarray(logits[sl]),
                "u": np.ascontiguousarray(U[sl]),
                "tidx": (np.arange(R, dtype=np.int64) * C + t).astype(np.int32),
                "tgt": t.astype(np.int32),
                "cb": np.ascontiguousarray(class_bias),
            }
        )
    return in_maps


def run(inputs, trace=False, **spmd_kwargs):
    logits = np.asarray(inputs["logits"], dtype=np.float32)
    targets = np.asarray(inputs["targets"])
    U = np.asarray(inputs["U"], dtype=np.float32)
    class_bias = np.asarray(inputs["class_bias"], dtype=np.float32)
    N, C = logits.shape

    nc = _get_nc(N // N_CORES, C)
    in_maps = make_in_maps(logits, targets, U, class_bias)
    res = run_bass_kernel_spmd(
        nc, in_maps, core_ids=list(range(N_CORES)), trace=trace, **spmd_kwargs
    )
    nll = np.stack([r["nll"] for r in res.results])  # [n_cores, 128, nblk]
    loss = np.float32(nll.sum(dtype=np.float64) / N)
    return loss, res


def kernel(**inputs):
    loss, _ = run(inputs)
    return loss


# revision 9
# speedup vs baseline: 1.2009x; 1.2009x over previous
"""Energy-based debias loss kernel for Trainium2 (8 NeuronCores, Bass/Tile).

Math (per row i of logits [N, C], with uniform noise U, class bias cb):
    S_i    = sum_j exp(L_ij)                      (logsumexp, no max-sub needed:
                                                   L ~ N(0,1), sums are safe fp32)
    lse_i  = ln(S_i)
    S'_i   = S_i - exp(L_it)                      (exclude-target sum)
    blse_i = ln(S'_i)
    beta_i = relu(blse_i) / lse_i                 (== where(-blse>0,0,-blse)/(-lse))
    g_ij   = -ln(-ln(U_ij + 1e-10) + 1e-10)       (gumbel from uniform)
    z_ij   = L_ij + beta_i * g_ij + ln(cb_j + 1e-12)
    nll_i  = ln(sum_j exp(z_ij)) - z_it
    loss   = mean_i nll_i

Engine mapping per 128-row block (rows on partitions, C streamed in chunks):
  pass 1 (per chunk):  ACT exp(L) with fused accum_out -> S partial sums
                       PE  ones[1,128] x lb[1,ck] -> PSUM broadcast of ln(cb)
                       DVE L += lb  (PSUM operand)
  pass 1.5 (tiny):     beta chain on [128,1] tiles; target gathers done upfront
                       via indirect DMA
  pass 2 (per chunk):  ACT ln(U+eps); ACT ln(-a+eps); DVE scalar_tensor_tensor
                       w = b*(-beta) + (L+lb); ACT exp(w) with fused accum -> S2
No standalone reduce passes; row sums ride the ACT accumulator.
"""

import numpy as np

import concourse.bass as bass
import concourse.bacc as bacc
import concourse.tile as tile
from concourse import mybir
from concourse.bass_utils import run_bass_kernel_spmd

P = 128
N_CORES = 8

# tunables
CK = 3200          # chunk size along C
L_BUFS_EXTRA = 0   # extra L slots beyond nch for cross-block overlap
U_BUFS = 3
MM_N = 512         # psum bank free size for the broadcast matmul

F32 = mybir.dt.float32
BF16 = mybir.dt.bfloat16
I32 = mybir.dt.int32
AF = mybir.ActivationFunctionType
ALU = mybir.AluOpType

_orig_get_activation_tables = bacc.get_activation_tables


def _combined_only_tables(arch):
    """Restrict the act-table pass to the set holding BOTH exp and ln
    (natural_log_exp_and_others), keeping list positions so
    act_func_set_id still indexes act_info.json correctly. Without this,
    bacc picks exp_and_others / natural_log alternately and the kernel
    pays ~1.3us ACT_TABLE_LOAD per Exp<->Ln switch (89 loads = 114us)."""
    t = _orig_get_activation_tables(arch)
    return {
        name: (fns if (AF.Exp in fns and AF.Ln in fns) else set())
        for name, fns in t.items()
    }


def build_nc(R, C, ck=CK):
    """Build the SPMD per-core program. R rows per core, C classes."""
    assert R % P == 0 and C % ck == 0 and C % P == 0
    nblk = R // P
    nch = C // ck
    ckb = C // P  # free size of the [128, C/128] class-bias view

    nc = bacc.Bacc(None, target_bir_lowering=False, debug=False)

    logits_in = nc.dram_tensor("logits", [R, C], F32, kind="ExternalInput")
    u_in = nc.dram_tensor("u", [R, C], F32, kind="ExternalInput")
    tidx_in = nc.dram_tensor("tidx", [R], I32, kind="ExternalInput")  # i*C+t_i
    tgt_in = nc.dram_tensor("tgt", [R], I32, kind="ExternalInput")    # t_i
    cb_in = nc.dram_tensor("cb", [C], F32, kind="ExternalInput")
    nll_out = nc.dram_tensor("nll", [P, nblk], F32, kind="ExternalOutput")
    lb_dram = nc.dram_tensor("lb_bf16", [C], BF16)  # internal scratch

    logits_flat = logits_in[:].rearrange("r c -> (r c)").unsqueeze(1)
    u_flat = u_in[:].rearrange("r c -> (r c)").unsqueeze(1)
    cb_flat = cb_in[:].unsqueeze(1)

    with tile.TileContext(nc) as tc:
        with (
            tc.tile_pool(name="consts", bufs=1) as consts,
            tc.tile_pool(name="Lpool", bufs=nch + L_BUFS_EXTRA) as lpool,
            tc.tile_pool(name="Upool", bufs=U_BUFS) as upool,
            tc.tile_pool(name="scr", bufs=1) as scrpool,
            tc.tile_pool(name="lbk", bufs=2) as lbkpool,
            tc.tile_pool(name="stats", bufs=4) as stats,
            tc.tile_pool(name="smalls", bufs=24) as smalls,
            tc.tile_pool(name="psum", bufs=1, space="PSUM") as psum,
        ):
            # ---- phase 0: constants, ln(class_bias), upfront gathers ----
            eps10 = consts.tile([P, 1], F32)  # 1e-10 bias for Ln
            nc.vector.memset(eps10[:], 1e-10)
            eps12 = consts.tile([P, 1], F32)  # 1e-12 bias for Ln(class_bias)
            nc.vector.memset(eps12[:], 1e-12)

            cb_t = consts.tile([P, ckb], F32)
            nc.sync.dma_start(out=cb_t[:], in_=cb_in[:].rearrange("(p k) -> p k", p=P))
            lb128 = consts.tile([P, ckb], BF16)
            nc.scalar.activation(out=lb128[:], in_=cb_t[:], func=AF.Ln, bias=eps12[:])
            nc.sync.dma_start(
                out=lb_dram[:].rearrange("(p k) -> p k", p=P), in_=lb128[:]
            )

            ones_bf = consts.tile([1, P], BF16)
            nc.vector.memset(ones_bf[:], 1.0)

            tidx_sb = consts.tile([P, nblk], I32)
            nc.sync.dma_start(
                out=tidx_sb[:], in_=tidx_in[:].rearrange("(b p) -> p b", p=P)
            )
            tgt_sb = consts.tile([P, nblk], I32)
            nc.sync.dma_start(
                out=tgt_sb[:], in_=tgt_in[:].rearrange("(b p) -> p b", p=P)
            )

            xt_all = consts.tile([P, nblk], F32)   # logits[i, t_i]
            ut_all = consts.tile([P, nblk], F32)   # U[i, t_i]
            cbt_all = consts.tile([P, nblk], F32)  # cb[t_i]
            for b in range(nblk):
                nc.gpsimd.indirect_dma_start(
                    out=xt_all[:, b : b + 1],
                    out_offset=None,
                    in_=logits_flat,
                    in_offset=bass.IndirectOffsetOnAxis(
                        ap=tidx_sb[:, b : b + 1], axis=0
                    ),
                )
                nc.gpsimd.indirect_dma_start(
                    out=ut_all[:, b : b + 1],
                    out_offset=None,
                    in_=u_flat,
                    in_offset=bass.IndirectOffsetOnAxis(
                        ap=tidx_sb[:, b : b + 1], axis=0
                    ),
                )
                nc.gpsimd.indirect_dma_start(
                    out=cbt_all[:, b : b + 1],
                    out_offset=None,
                    in_=cb_flat,
                    in_offset=bass.IndirectOffsetOnAxis(
                        ap=tgt_sb[:, b : b + 1], axis=0
                    ),
                )

            # target-side tiny precomputes (block independent)
            eT_all = consts.tile([P, nblk], F32)
            nc.scalar.activation(out=eT_all[:], in_=xt_all[:], func=AF.Exp)
            at_all = consts.tile([P, nblk], F32)
            nc.scalar.activation(out=at_all[:], in_=ut_all[:], func=AF.Ln, bias=eps10[:])
            bt_all = consts.tile([P, nblk], F32)
            nc.scalar.activation(
                out=bt_all[:], in_=at_all[:], func=AF.Ln, scale=-1.0, bias=eps10[:]
            )
            lbt_all = consts.tile([P, nblk], F32)
            nc.scalar.activation(out=lbt_all[:], in_=cbt_all[:], func=AF.Ln, bias=eps12[:])
            s1_all = consts.tile([P, nblk], F32)  # x_t + ln(cb_t)
            nc.vector.tensor_tensor(
                out=s1_all[:], in0=xt_all[:], in1=lbt_all[:], op=ALU.add
            )

            nll_sb = consts.tile([P, nblk], F32)

            for b in range(nblk):
                r0 = b * P
                s_part = stats.tile([P, nch], F32, tag="spart")
                s2_part = stats.tile([P, nch], F32, tag="s2part")
                l_tiles = []

                # ---- pass 1: S row-sums + fold ln(cb) into cached L ----
                for c in range(nch):
                    c0 = c * ck
                    lt = lpool.tile([P, ck], F32, tag="L")
                    l_tiles.append(lt)
                    nc.sync.dma_start(
                        out=lt[:], in_=logits_in[r0 : r0 + P, c0 : c0 + ck]
                    )
                    scr = scrpool.tile([P, ck], F32, tag="scr")
                    nc.scalar.activation(
                        out=scr[:],
                        in_=lt[:],
                        func=AF.Exp,
                        accum_out=s_part[:, c : c + 1],
                    )
                    lbk = lbkpool.tile([1, ck], BF16, tag="lbk")
                    nc.sync.dma_start(
                        out=lbk[:],
                        in_=lb_dram[c0 : c0 + ck].rearrange("(a c) -> a c", a=1),
                    )
                    lbb = psum.tile([P, ck], F32, tag="lbb")
                    for j in range(0, ck, MM_N):
                        n = min(MM_N, ck - j)
                        nc.tensor.matmul(
                            out=lbb[:, j : j + n],
                            lhsT=ones_bf[:],
                            rhs=lbk[:, j : j + n],
                            start=True,
                            stop=True,
                        )
                    nc.vector.tensor_tensor(
                        out=lt[:], in0=lt[:], in1=lbb[:], op=ALU.add
                    )

                # ---- pass 1.5: beta / z_t (tiny [128,1] chain) ----
                s_sum = smalls.tile([P, 1], F32, tag="sm")
                nc.vector.reduce_sum(
                    out=s_sum[:], in_=s_part[:], axis=mybir.AxisListType.X
                )
                lse = smalls.tile([P, 1], F32, tag="sm")
                nc.scalar.activation(out=lse[:], in_=s_sum[:], func=AF.Ln)
                sp = smalls.tile([P, 1], F32, tag="sm")
                nc.vector.tensor_tensor(
                    out=sp[:], in0=s_sum[:], in1=eT_all[:, b : b + 1], op=ALU.subtract
                )
                blse = smalls.tile([P, 1], F32, tag="sm")
                nc.scalar.activation(out=blse[:], in_=sp[:], func=AF.Ln)
                rb = smalls.tile([P, 1], F32, tag="sm")
                nc.scalar.activation(out=rb[:], in_=blse[:], func=AF.Relu)
                invl = smalls.tile([P, 1], F32, tag="sm")
                nc.vector.reciprocal(out=invl[:], in_=lse[:])
                beta = smalls.tile([P, 1], F32, tag="sm")
                nc.vector.tensor_tensor(out=beta[:], in0=rb[:], in1=invl[:], op=ALU.mult)
                negbeta = smalls.tile([P, 1], F32, tag="sm")
                nc.vector.tensor_scalar_mul(out=negbeta[:], in0=beta[:], scalar1=-1.0)
                znt = smalls.tile([P, 1], F32, tag="sm")  # beta * b_t
                nc.vector.tensor_tensor(
                    out=znt[:], in0=bt_all[:, b : b + 1], in1=beta[:], op=ALU.mult
                )
                zt = smalls.tile([P, 1], F32, tag="sm")  # z at target
                nc.vector.tensor_tensor(
                    out=zt[:], in0=s1_all[:, b : b + 1], in1=znt[:], op=ALU.subtract
                )

                # ---- pass 2: S2 = sum_j exp(L + lb - beta*b) ----
                for c in range(nch):
                    c0 = c * ck
                    ut = upool.tile([P, ck], F32, tag="U")
                    nc.sync.dma_start(out=ut[:], in_=u_in[r0 : r0 + P, c0 : c0 + ck])
                    nc.scalar.activation(
                        out=ut[:], in_=ut[:], func=AF.Ln, bias=eps10[:]
                    )
                    nc.scalar.activation(
                        out=ut[:], in_=ut[:], func=AF.Ln, scale=-1.0, bias=eps10[:]
                    )
                    nc.vector.scalar_tensor_tensor(
                        out=ut[:],
                        in0=ut[:],
                        scalar=negbeta[:],
                        op0=ALU.mult,
                        in1=l_tiles[c][:],
                        op1=ALU.add,
                    )
                    nc.scalar.activation(
                        out=ut[:],
                        in_=ut[:],
                        func=AF.Exp,
                        accum_out=s2_part[:, c : c + 1],
                    )

                s2_sum = smalls.tile([P, 1], F32, tag="sm")
                nc.vector.reduce_sum(
                    out=s2_sum[:], in_=s2_part[:], axis=mybir.AxisListType.X
                )
                l2 = smalls.tile([P, 1], F32, tag="sm")
                nc.scalar.activation(out=l2[:], in_=s2_sum[:], func=AF.Ln)
                nc.vector.tensor_tensor(
                    out=nll_sb[:, b : b + 1], in0=l2[:], in1=zt[:], op=ALU.subtract
                )

            nc.sync.dma_start(out=nll_out[:], in_=nll_sb[:])

    bacc.get_activation_tables = _combined_only_tables
    try:
        nc.finalize()  # runs Bacc.compile(): register alloc, act-table loads
    finally:
        bacc.get_activation_tables = _orig_get_activation_tables
    return nc


_NC_CACHE = {}


def _get_nc(R, C, ck=CK):
    key = (R, C, ck)
    if key not in _NC_CACHE:
        _NC_CACHE[key] = build_nc(R, C, ck)
    return _NC_CACHE[key]


def make_in_maps(logits, targets, U, class_bias, n_cores=N_CORES):
    N, C = logits.shape
    R = N // n_cores
    in_maps = []
    for k in range(n_cores):
        sl = slice(k * R, (k + 1) * R)
        t = np.asarray(targets[sl], dtype=np.int64)
        in_maps.append(
            {
                "logits": np.ascontiguousarray(logits[sl]),
                "u": np.ascontiguousarray(U[sl]),
                "tidx": (np.arange(R, dtype=np.int64) * C + t).astype(np.int32),
                "tgt": t.astype(np.int32),
                "cb": np.ascontiguousarray(class_bias),
            }
        )
    return in_maps


def run(inputs, trace=False, **spmd_kwargs):
    logits = np.asarray(inputs["logits"], dtype=np.float32)
    targets = np.asarray(inputs["targets"])
    U = np.asarray(inputs["U"], dtype=np.float32)
    class_bias = np.asarray(inputs["class_bias"], dtype=np.float32)
    N, C = logits.shape

    nc = _get_nc(N // N_CORES, C)
    in_maps = make_in_maps(logits, targets, U, class_bias)
    res = run_bass_kernel_spmd(
        nc, in_maps, core_ids=list(range(N_CORES)), trace=trace, **spmd_kwargs
    )
    nll = np.stack([r["nll"] for r in res.results])  # [n_cores, 128, nblk]
    loss = np.float32(nll.sum(dtype=np.float64) / N)
    return loss, res


def kernel(**inputs):
    loss, _ = run(inputs)
    return loss
